# revision 22
# baseline (speedup 1.0000x reference)
"""Trainium2 Bass kernel for nn_LNKillingRelu (fp16 pipeline).

Math (per batch b, channel g, point n; L=8 lie-algebra coords):
    d[b,g,:,n]  = sum_f W[g,f] * x[b,f,:,n]          (64x64 linear over channels)
    kf[b,g,n]   = x[b,g,:,n]^T K d[b,g,:,n]          (8x8 Killing bilinear form)
    out         = x + relu(kf) * d                    (broadcast kf over L)

K is symmetric & sparse: pairs (0<->6, 1<->7, 2<->3) plus [[2,1],[1,2]] on
coords (4,5).  We use kf = sum_l (Kx)[l] * d[l] with
(Kx) = [x6, x7, x3, x2, 2x4+x5, x4+2x5, x0, x1].

Everything on-device runs in fp16 (tolerance is 2e-2 scale-relative; fp16
keeps it ~1e-3): halves DMA bytes (the binding resource - all DMA transfers
serialize at ~360B/ns aggregate), doubles DVE throughput (2x packed mode),
and runs the PE at 1 cycle/row instead of 4.  Host converts fp32<->fp16.

Sharding: data-parallel over batch B=16 -> 2 batches per core (8 cores).
Per-core layout: partitions = (batch-pair, F=64) = 128; free = (L, n-chunk).
PE computes d with a block-diag(W^T, W^T) 128x128 stationary fp16 weight.
ACT drains PSUM->SBUF (fp32->fp16); DVE+GPSIMD share the Killing products,
tree-reduce, and the out = x + relu(kf)*d tail.

Software pipelining: relu(k-1) and the out-stage(k-1) are emitted inside
chunk k - relu goes on ACT *before* chunk k's PSUM drains (so ACT never
stalls on the DVE kf-chain), and the out-stage fills the DVE/Pool gap while
chunk k's D is still in the PE/ACT pipe.  Chunk sizes ramp 256 -> 512 -> 256
to shorten the pipeline head/tail; all tiles are allocated at max width and
column-sliced so the tile pools see a single tag.
"""

import numpy as np
from contextlib import ExitStack

import concourse.bass as bass
import concourse.bacc as bacc
import concourse.tile as tile
from concourse import mybir
from concourse.bass_utils import run_bass_kernel_spmd

B, F, L, N = 16, 64, 8, 4096
N_CORES = 8
B_PER_CORE = B // N_CORES  # 2

F16 = mybir.dt.float16
F32 = mybir.dt.float32
MULT = mybir.AluOpType.mult
ADD = mybir.AluOpType.add

NQ = 256  # psum quarter width (4 banks fp32 at L=8)


def _bcast_l(ap: bass.AP, l: int) -> bass.AP:
    """[128, NT] AP -> [128, l, NT] with a zero-stride middle dim."""
    return bass.AP(tensor=ap.tensor, offset=ap.offset,
                   ap=[ap.ap[0], [0, l], ap.ap[1]])


# default engine assignment: 'v' = DVE, 'p' = GPSIMD(Pool)
DEFAULT_ASSIGN = dict(pa='v', pb='v', pc='p', pd='v', pe='v',
                      xe0='p', xe1='p', l1='v', l2='v', l3='v',
                      osplit=3, pipe=1, ocolsplit=1, copygrain='pair',
                      pdsplit=0, relu='a')
DEFAULT_CHUNKS = (256, 256, 512, 512, 512, 512, 512, 512, 256, 256)


def _build(chunks=DEFAULT_CHUNKS, assign=None):
    asn = dict(DEFAULT_ASSIGN)
    if assign:
        asn.update(assign)
    assert sum(chunks) == N
    ntmax = max(chunks)
    nc = bacc.Bacc("TRN2", target_bir_lowering=False, debug=False,
                   num_devices=N_CORES)
    x = nc.dram_tensor("x", [B_PER_CORE, F, L, N], F16, kind="ExternalInput").ap()
    w2t = nc.dram_tensor("w2t", [128, 128], F16, kind="ExternalInput").ap()
    out = nc.dram_tensor("out", [B_PER_CORE, F, L, N], F16, kind="ExternalOutput").ap()

    xv = x.rearrange("b f l n -> (b f) l n")    # [128, 8, N]
    ov = out.rearrange("b f l n -> (b f) l n")

    def eng(key):
        return nc.vector if asn[key] == 'v' else nc.gpsimd

    with ExitStack() as ctx:
        tc = ctx.enter_context(tile.TileContext(nc))
        singles = ctx.enter_context(tc.tile_pool(name="singles", bufs=1))
        xpool = ctx.enter_context(tc.tile_pool(name="xp", bufs=3))
        dpool = ctx.enter_context(tc.tile_pool(name="dp", bufs=3))
        vpool = ctx.enter_context(tc.tile_pool(name="vp", bufs=2))
        tpool = ctx.enter_context(tc.tile_pool(name="tp", bufs=2))
        mpool = ctx.enter_context(tc.tile_pool(name="mp", bufs=2))
        opool = ctx.enter_context(tc.tile_pool(name="op", bufs=3))
        rpool = ctx.enter_context(tc.tile_pool(name="rp", bufs=3))
        psum_bufs = {'pair': 4, 'half': 2}[asn['copygrain']]
        psum = ctx.enter_context(tc.tile_pool(name="ps", bufs=psum_bufs,
                                              space="PSUM"))

        w_sb = singles.tile([128, 128], F16)
        nc.sync.dma_start(out=w_sb[:], in_=w2t)

        pending = []  # deferred tail: (X, D, kf, n0, nt)

        def emit_relu(item):
            X, D, kf, n0, nt = item
            r = rpool.tile([128, ntmax], F16, tag="r")
            if asn['relu'] == 'a':
                nc.scalar.activation(r[:, 0:nt], kf[:, 0:nt],
                                     mybir.ActivationFunctionType.Relu)
            else:
                e = nc.vector if asn['relu'] == 'v' else nc.gpsimd
                e.tensor_scalar_max(r[:, 0:nt], kf[:, 0:nt], 0.0)
            return (X, D, r, n0, nt)

        def emit_out_stage(item):
            X, D, r, n0, nt = item
            M = mpool.tile([128, L, ntmax], F16, tag="M")
            O = opool.tile([128, L, ntmax], F16, tag="O")
            s = asn['osplit']
            nh = nt // 2
            # Pool's share (and the out-DMA) go in column halves so the first
            # half can enter the serialized DMA queue while the second is
            # still being computed.  DVE's share is one op (it finishes early).
            colsplit = asn['ocolsplit'] and s < L and nh >= NQ
            if s > 0:
                nc.vector.tensor_mul(M[:, 0:s, 0:nt], D[:, 0:s, 0:nt],
                                     _bcast_l(r[:, 0:nt], s))
                nc.vector.tensor_add(O[:, 0:s, 0:nt], X[:, 0:s, 0:nt],
                                     M[:, 0:s, 0:nt])
            pieces = [(0, nh), (nh, nt)] if colsplit else [(0, nt)]
            for (a, b) in pieces:
                if s < L:
                    rb = _bcast_l(r[:, a:b], L - s)
                    nc.gpsimd.tensor_mul(M[:, s:L, a:b], D[:, s:L, a:b], rb)
                    nc.gpsimd.tensor_add(O[:, s:L, a:b], X[:, s:L, a:b],
                                         M[:, s:L, a:b])
                nc.sync.dma_start(out=ov[:, :, n0 + a:n0 + b], in_=O[:, :, a:b])

        n0 = 0
        for nt in chunks:
            # in-DMA split by column halves: the first half's matmuls can
            # start while the second half is still transferring.
            X = xpool.tile([128, L, ntmax], F16, tag="X")
            nh = nt // 2
            if nh >= NQ:  # below 256 cols the 512B-descriptor rate halves
                nc.sync.dma_start(out=X[:, :, 0:nh], in_=xv[:, :, n0:n0 + nh])
                nc.sync.dma_start(out=X[:, :, nh:nt], in_=xv[:, :, n0 + nh:n0 + nt])
            else:
                nc.sync.dma_start(out=X[:, :, 0:nt], in_=xv[:, :, n0:n0 + nt])

            # Xe = (2x4+x5, x4+2x5).  GPSIMD has no TensorScalarPtr, so the
            # Pool path uses plain adds via s = x4+x5; the DVE path can use
            # scalar_tensor_tensor directly (full rate but one op per plane).
            # Needs only X, fills the engines early.
            Xe = tpool.tile([128, 2, ntmax], F16, tag="Xe")
            if asn['xe0'] == 'v' and asn['xe1'] == 'v' and asn.get('xestt', 1):
                nc.vector.scalar_tensor_tensor(
                    Xe[:, 0, 0:nt], in0=X[:, 4, 0:nt], scalar=2.0,
                    in1=X[:, 5, 0:nt], op0=MULT, op1=ADD)
                nc.vector.scalar_tensor_tensor(
                    Xe[:, 1, 0:nt], in0=X[:, 5, 0:nt], scalar=2.0,
                    in1=X[:, 4, 0:nt], op0=MULT, op1=ADD)
            else:
                xs = tpool.tile([128, ntmax], F16, tag="xs")
                eng('xe0').tensor_add(xs[:, 0:nt], X[:, 4, 0:nt], X[:, 5, 0:nt])
                eng('xe0').tensor_add(Xe[:, 0, 0:nt], xs[:, 0:nt], X[:, 4, 0:nt])
                eng('xe1').tensor_add(Xe[:, 1, 0:nt], xs[:, 0:nt], X[:, 5, 0:nt])

            # relu of the previous chunk: on ACT *before* this chunk's
            # copies, so it never blocks them (its input is long done).
            prev = None
            if pending:
                prev = emit_relu(pending.pop(0))

            # D = W @ X.  PSUM tile height = ACT copy granularity g: finer
            # (g=2) unblocks downstream products earlier, coarser (g=4) cuts
            # ACT per-op overhead.  g*ntmax/256 banks per tile, 8 banks total.
            g = {'pair': 2, 'half': 4}[asn['copygrain']]
            D = dpool.tile([128, L, ntmax], F16, tag="D")
            for p in range(L // g):
                ps = psum.tile([128, g, ntmax], F32, tag="ps")
                for k in range(g):
                    nc.tensor.matmul(ps[:, k, 0:nt], lhsT=w_sb[:],
                                     rhs=X[:, g * p + k, 0:nt],
                                     start=True, stop=True)
                nc.scalar.copy(D[:, g * p:g * (p + 1), 0:nt], ps[:, :, 0:nt])

            # deferred out-stage of the previous chunk: ready to run, fills
            # the DVE/Pool gap while this chunk's D is in the PE/ACT pipe.
            if prev is not None:
                emit_out_stage(prev)

            # V[l] = (Kx)[l] * D[l]
            V = vpool.tile([128, L, ntmax], F16, tag="V")
            eng('pa').tensor_mul(V[:, 0:2, 0:nt], X[:, 6:8, 0:nt], D[:, 0:2, 0:nt])
            eng('pb').tensor_mul(V[:, 2, 0:nt], X[:, 3, 0:nt], D[:, 2, 0:nt])
            eng('pc').tensor_mul(V[:, 3, 0:nt], X[:, 2, 0:nt], D[:, 3, 0:nt])
            if asn['pdsplit']:
                eng('pd').tensor_mul(V[:, 4, 0:nt], Xe[:, 0, 0:nt], D[:, 4, 0:nt])
                eng('pd').tensor_mul(V[:, 5, 0:nt], Xe[:, 1, 0:nt], D[:, 5, 0:nt])
            else:
                eng('pd').tensor_mul(V[:, 4:6, 0:nt], Xe[:, :, 0:nt],
                                     D[:, 4:6, 0:nt])
            eng('pe').tensor_mul(V[:, 6:8, 0:nt], X[:, 0:2, 0:nt], D[:, 6:8, 0:nt])

            # kf = sum_l V[l] via tree adds (tensor_tensor keeps the fp16 2x
            # DVE mode; tensor_reduce over strided l would run at full rate)
            T4 = tpool.tile([128, 4, ntmax], F16, tag="T4")
            T2 = tpool.tile([128, 2, ntmax], F16, tag="T2")
            kf = rpool.tile([128, ntmax], F16, tag="kf")
            eng('l1').tensor_add(T4[:, :, 0:nt], V[:, 0:4, 0:nt], V[:, 4:8, 0:nt])
            eng('l2').tensor_add(T2[:, :, 0:nt], T4[:, 0:2, 0:nt], T4[:, 2:4, 0:nt])
            eng('l3').tensor_add(kf[:, 0:nt], T2[:, 0, 0:nt], T2[:, 1, 0:nt])

            if asn['pipe']:
                pending.append((X, D, kf, n0, nt))
            else:
                emit_out_stage(emit_relu((X, D, kf, n0, nt)))
            n0 += nt

        for item in pending:
            emit_out_stage(emit_relu(item))

    nc.finalize()
    return nc


_CACHED = {}
CFG = (DEFAULT_CHUNKS, ())


def _freeze(cfg):
    chunks, assign = cfg
    return (tuple(chunks), tuple(sorted(dict(assign).items())))


def _get_program(cfg=None):
    cfg = cfg or CFG
    key = _freeze(cfg)
    if key not in _CACHED:
        _CACHED[key] = _build(tuple(cfg[0]), dict(cfg[1]))
    return _CACHED[key]


def _run(x: np.ndarray, W: np.ndarray, trace: bool = False, cfg=None):
    nc = _get_program(cfg)
    w2t = np.zeros((128, 128), dtype=np.float16)
    wt = np.ascontiguousarray(W.T).astype(np.float16)
    w2t[:64, :64] = wt
    w2t[64:, 64:] = wt
    x16 = np.asarray(x, dtype=np.float16)
    in_maps = [
        {"x": np.ascontiguousarray(x16[c * B_PER_CORE:(c + 1) * B_PER_CORE]),
         "w2t": w2t}
        for c in range(N_CORES)
    ]
    res = run_bass_kernel_spmd(nc, in_maps, list(range(N_CORES)), trace=trace)
    out = np.concatenate([res.results[c]["out"] for c in range(N_CORES)], axis=0)
    return out.astype(np.float32), res


def kernel(x: np.ndarray, W: np.ndarray) -> np.ndarray:
    out, _ = _run(np.asarray(x, dtype=np.float32), np.asarray(W, dtype=np.float32))
    return out


# revision 28
# speedup vs baseline: 1.0347x; 1.0347x over previous
"""Trainium2 Bass kernel for nn_LNKillingRelu (fp16 pipeline).

Math (per batch b, channel g, point n; L=8 lie-algebra coords):
    d[b,g,:,n]  = sum_f W[g,f] * x[b,f,:,n]          (64x64 linear over channels)
    kf[b,g,n]   = x[b,g,:,n]^T K d[b,g,:,n]          (8x8 Killing bilinear form)
    out         = x + relu(kf) * d                    (broadcast kf over L)

K is symmetric & sparse: pairs (0<->6, 1<->7, 2<->3) plus [[2,1],[1,2]] on
coords (4,5).  We use kf = sum_l (Kx)[l] * d[l] with
(Kx) = [x6, x7, x3, x2, 2x4+x5, x4+2x5, x0, x1].

Everything on-device runs in fp16 (tolerance is 2e-2 scale-relative; fp16
keeps it ~1e-3): halves DMA bytes (the binding resource - all DMA transfers
serialize at ~360B/ns aggregate), doubles DVE throughput (2x packed mode),
and runs the PE at 1 cycle/row instead of 4.  Host converts fp32<->fp16.

Sharding: data-parallel over batch B=16 -> 2 batches per core (8 cores).
Per-core layout: partitions = (batch-pair, F=64) = 128; free = (L, n-chunk).
PE computes d with a block-diag(W^T, W^T) 128x128 stationary fp16 weight.
ACT drains PSUM->SBUF (fp32->fp16); DVE+GPSIMD share the Killing products,
tree-reduce, and the out = x + relu(kf)*d tail.

Software pipelining: relu(k-1) and the out-stage(k-1) are emitted inside
chunk k - relu goes on ACT *before* chunk k's PSUM drains (so ACT never
stalls on the DVE kf-chain), and the out-stage fills the DVE/Pool gap while
chunk k's D is still in the PE/ACT pipe.  Chunk sizes ramp 256 -> 512 -> 256
to shorten the pipeline head/tail; all tiles are allocated at max width and
column-sliced so the tile pools see a single tag.
"""

import numpy as np
from contextlib import ExitStack

import concourse.bass as bass
import concourse.bacc as bacc
import concourse.tile as tile
from concourse import mybir
from concourse.bass_utils import run_bass_kernel_spmd

B, F, L, N = 16, 64, 8, 4096
N_CORES = 8
B_PER_CORE = B // N_CORES  # 2

F16 = mybir.dt.float16
F32 = mybir.dt.float32
MULT = mybir.AluOpType.mult
ADD = mybir.AluOpType.add

NQ = 256  # psum quarter width (4 banks fp32 at L=8)


def _bcast_l(ap: bass.AP, l: int) -> bass.AP:
    """[128, NT] AP -> [128, l, NT] with a zero-stride middle dim."""
    return bass.AP(tensor=ap.tensor, offset=ap.offset,
                   ap=[ap.ap[0], [0, l], ap.ap[1]])


# default engine assignment: 'v' = DVE, 'p' = GPSIMD(Pool)
DEFAULT_ASSIGN = dict(pa='v', pb='v', pc='p', pd='v', pe='v',
                      xe0='p', xe1='p', l1='v', l2='v', l3='v',
                      osplit=3, pipe=1, ocolsplit=1, copygrain='pair',
                      pdsplit=0, relu='a', merge_pape=0, merge_pbpc=0)
DEFAULT_CHUNKS = (256, 256, 512, 512, 512, 512, 512, 512, 256, 256)


def _build(chunks=DEFAULT_CHUNKS, assign=None):
    asn = dict(DEFAULT_ASSIGN)
    if assign:
        asn.update(assign)
    assert sum(chunks) == N
    ntmax = max(chunks)
    nc = bacc.Bacc("TRN2", target_bir_lowering=False, debug=False,
                   num_devices=N_CORES)
    x = nc.dram_tensor("x", [B_PER_CORE, F, L, N], F16, kind="ExternalInput").ap()
    w2t = nc.dram_tensor("w2t", [128, 128], F16, kind="ExternalInput").ap()
    out = nc.dram_tensor("out", [B_PER_CORE, F, L, N], F16, kind="ExternalOutput").ap()

    xv = x.rearrange("b f l n -> (b f) l n")    # [128, 8, N]
    ov = out.rearrange("b f l n -> (b f) l n")

    def eng(key):
        return nc.vector if asn[key] == 'v' else nc.gpsimd

    with ExitStack() as ctx:
        tc = ctx.enter_context(tile.TileContext(nc))
        singles = ctx.enter_context(tc.tile_pool(name="singles", bufs=1))
        xpool = ctx.enter_context(tc.tile_pool(name="xp", bufs=3))
        dpool = ctx.enter_context(tc.tile_pool(name="dp", bufs=3))
        vpool = ctx.enter_context(tc.tile_pool(name="vp", bufs=2))
        tpool = ctx.enter_context(tc.tile_pool(name="tp", bufs=2))
        mpool = ctx.enter_context(tc.tile_pool(name="mp", bufs=2))
        opool = ctx.enter_context(tc.tile_pool(name="op", bufs=3))
        rpool = ctx.enter_context(tc.tile_pool(name="rp", bufs=3))
        psum_bufs = {'pair': 4, 'half': 2}[asn['copygrain']]
        psum = ctx.enter_context(tc.tile_pool(name="ps", bufs=psum_bufs,
                                              space="PSUM"))

        w_sb = singles.tile([128, 128], F16)
        nc.sync.dma_start(out=w_sb[:], in_=w2t)
        zero16 = singles.tile([128, 1], F16, tag="zero16")
        nc.vector.memset(zero16[:], 0.0)

        pending = []  # deferred tail: (X, D, kf, n0, nt)

        def emit_relu(item):
            X, D, kf, n0, nt = item
            r = rpool.tile([128, ntmax], F16, tag="r")
            if asn['relu'] == 'a':
                nc.scalar.activation(r[:, 0:nt], kf[:, 0:nt],
                                     mybir.ActivationFunctionType.Relu)
            elif asn['relu'] == 'v4':
                # fp16 [P,1] AP scalar keeps the 4x_2p DVE mode (a float
                # immediate would be fp32 and disqualify it)
                nc.vector.tensor_scalar_max(r[:, 0:nt], kf[:, 0:nt],
                                            zero16[:])
            else:
                e = nc.vector if asn['relu'] == 'v' else nc.gpsimd
                e.tensor_scalar_max(r[:, 0:nt], kf[:, 0:nt], 0.0)
            return (X, D, r, n0, nt)

        def emit_out_stage(item):
            X, D, r, n0, nt = item
            M = mpool.tile([128, L, ntmax], F16, tag="M")
            O = opool.tile([128, L, ntmax], F16, tag="O")
            s = asn['osplit']
            nh = nt // 2
            # Pool's share (and the out-DMA) go in column halves so the first
            # half can enter the serialized DMA queue while the second is
            # still being computed.  DVE's share is one op (it finishes early).
            colsplit = asn['ocolsplit'] and s < L and nh >= NQ
            if s > 0:
                nc.vector.tensor_mul(M[:, 0:s, 0:nt], D[:, 0:s, 0:nt],
                                     _bcast_l(r[:, 0:nt], s))
                nc.vector.tensor_add(O[:, 0:s, 0:nt], X[:, 0:s, 0:nt],
                                     M[:, 0:s, 0:nt])
            pieces = [(0, nh), (nh, nt)] if colsplit else [(0, nt)]
            for (a, b) in pieces:
                if s < L:
                    rb = _bcast_l(r[:, a:b], L - s)
                    nc.gpsimd.tensor_mul(M[:, s:L, a:b], D[:, s:L, a:b], rb)
                    nc.gpsimd.tensor_add(O[:, s:L, a:b], X[:, s:L, a:b],
                                         M[:, s:L, a:b])
                nc.sync.dma_start(out=ov[:, :, n0 + a:n0 + b], in_=O[:, :, a:b])

        n0 = 0
        for nt in chunks:
            # in-DMA split by column halves: the first half's matmuls can
            # start while the second half is still transferring.
            X = xpool.tile([128, L, ntmax], F16, tag="X")
            nh = nt // 2
            if nh >= NQ:  # below 256 cols the 512B-descriptor rate halves
                nc.sync.dma_start(out=X[:, :, 0:nh], in_=xv[:, :, n0:n0 + nh])
                nc.sync.dma_start(out=X[:, :, nh:nt], in_=xv[:, :, n0 + nh:n0 + nt])
            else:
                nc.sync.dma_start(out=X[:, :, 0:nt], in_=xv[:, :, n0:n0 + nt])

            # Xe = (2x4+x5, x4+2x5).  GPSIMD has no TensorScalarPtr, so the
            # Pool path uses plain adds via s = x4+x5; the DVE path can use
            # scalar_tensor_tensor directly (full rate but one op per plane).
            # Needs only X, fills the engines early.
            Xe = tpool.tile([128, 2, ntmax], F16, tag="Xe")
            if asn['xe0'] == 'v' and asn['xe1'] == 'v' and asn.get('xestt', 1):
                nc.vector.scalar_tensor_tensor(
                    Xe[:, 0, 0:nt], in0=X[:, 4, 0:nt], scalar=2.0,
                    in1=X[:, 5, 0:nt], op0=MULT, op1=ADD)
                nc.vector.scalar_tensor_tensor(
                    Xe[:, 1, 0:nt], in0=X[:, 5, 0:nt], scalar=2.0,
                    in1=X[:, 4, 0:nt], op0=MULT, op1=ADD)
            else:
                xs = tpool.tile([128, ntmax], F16, tag="xs")
                eng('xe0').tensor_add(xs[:, 0:nt], X[:, 4, 0:nt], X[:, 5, 0:nt])
                eng('xe0').tensor_add(Xe[:, 0, 0:nt], xs[:, 0:nt], X[:, 4, 0:nt])
                eng('xe1').tensor_add(Xe[:, 1, 0:nt], xs[:, 0:nt], X[:, 5, 0:nt])

            # relu of the previous chunk: on ACT *before* this chunk's
            # copies, so it never blocks them (its input is long done).
            prev = None
            if pending:
                prev = emit_relu(pending.pop(0))

            # D = W @ X.  PSUM tile height = ACT copy granularity g: finer
            # (g=2) unblocks downstream products earlier, coarser (g=4) cuts
            # ACT per-op overhead.  g*ntmax/256 banks per tile, 8 banks total.
            g = {'pair': 2, 'half': 4}[asn['copygrain']]
            D = dpool.tile([128, L, ntmax], F16, tag="D")
            for p in range(L // g):
                ps = psum.tile([128, g, ntmax], F32, tag="ps")
                for k in range(g):
                    nc.tensor.matmul(ps[:, k, 0:nt], lhsT=w_sb[:],
                                     rhs=X[:, g * p + k, 0:nt],
                                     start=True, stop=True)
                nc.scalar.copy(D[:, g * p:g * (p + 1), 0:nt], ps[:, :, 0:nt])

            # deferred out-stage of the previous chunk: ready to run, fills
            # the DVE/Pool gap while this chunk's D is in the PE/ACT pipe.
            if prev is not None:
                emit_out_stage(prev)

            # V[l] = (Kx)[l] * D[l]
            V = vpool.tile([128, L, ntmax], F16, tag="V")

            def _blk4(t, l0, blkstride, lstride_sign=1):
                """4D AP over tile t: [part, blk=2 (stride blkstride),
                l=2 (stride +-ls), nt] starting at plane l0."""
                base = t[:]
                pdim = base.ap[0]
                ls = ntmax
                return bass.AP(tensor=base.tensor,
                               offset=base.offset + l0 * ls,
                               ap=[pdim, [blkstride * ls, 2],
                                   [lstride_sign * ls, 2], [1, nt]])

            if asn['merge_pape']:
                # V[{0,1}],V[{6,7}] = X[{6,7}],X[{0,1}] * D[{0,1}],D[{6,7}]
                eng('pa').tensor_tensor(
                    _blk4(V, 0, 6), _blk4(X, 6, -6), _blk4(D, 0, 6), MULT)
            else:
                eng('pa').tensor_mul(V[:, 0:2, 0:nt], X[:, 6:8, 0:nt],
                                     D[:, 0:2, 0:nt])
            if asn['merge_pbpc']:
                # V[2:4] = X[{3,2}] * D[2:4] via negative l-stride on X
                xrev = bass.AP(tensor=X[:].tensor,
                               offset=X[:].offset + 3 * ntmax,
                               ap=[X[:].ap[0], [-ntmax, 2], [1, nt]])
                eng('pb').tensor_tensor(V[:, 2:4, 0:nt], xrev,
                                        D[:, 2:4, 0:nt], MULT)
            else:
                eng('pb').tensor_mul(V[:, 2, 0:nt], X[:, 3, 0:nt], D[:, 2, 0:nt])
                eng('pc').tensor_mul(V[:, 3, 0:nt], X[:, 2, 0:nt], D[:, 3, 0:nt])
            if asn['pdsplit']:
                eng('pd').tensor_mul(V[:, 4, 0:nt], Xe[:, 0, 0:nt], D[:, 4, 0:nt])
                eng('pd').tensor_mul(V[:, 5, 0:nt], Xe[:, 1, 0:nt], D[:, 5, 0:nt])
            else:
                eng('pd').tensor_mul(V[:, 4:6, 0:nt], Xe[:, :, 0:nt],
                                     D[:, 4:6, 0:nt])
            if not asn['merge_pape']:
                eng('pe').tensor_mul(V[:, 6:8, 0:nt], X[:, 0:2, 0:nt],
                                     D[:, 6:8, 0:nt])

            # kf = sum_l V[l] via tree adds (tensor_tensor keeps the fp16 2x
            # DVE mode; tensor_reduce over strided l would run at full rate)
            T4 = tpool.tile([128, 4, ntmax], F16, tag="T4")
            T2 = tpool.tile([128, 2, ntmax], F16, tag="T2")
            kf = rpool.tile([128, ntmax], F16, tag="kf")
            eng('l1').tensor_add(T4[:, :, 0:nt], V[:, 0:4, 0:nt], V[:, 4:8, 0:nt])
            eng('l2').tensor_add(T2[:, :, 0:nt], T4[:, 0:2, 0:nt], T4[:, 2:4, 0:nt])
            eng('l3').tensor_add(kf[:, 0:nt], T2[:, 0, 0:nt], T2[:, 1, 0:nt])

            if asn['pipe']:
                pending.append((X, D, kf, n0, nt))
            else:
                emit_out_stage(emit_relu((X, D, kf, n0, nt)))
            n0 += nt

        for item in pending:
            emit_out_stage(emit_relu(item))

    nc.finalize()
    return nc


_CACHED = {}
CFG = (DEFAULT_CHUNKS, ())


def _freeze(cfg):
    chunks, assign = cfg
    return (tuple(chunks), tuple(sorted(dict(assign).items())))


def _get_program(cfg=None):
    cfg = cfg or CFG
    key = _freeze(cfg)
    if key not in _CACHED:
        _CACHED[key] = _build(tuple(cfg[0]), dict(cfg[1]))
    return _CACHED[key]


def _run(x: np.ndarray, W: np.ndarray, trace: bool = False, cfg=None):
    nc = _get_program(cfg)
    w2t = np.zeros((128, 128), dtype=np.float16)
    wt = np.ascontiguousarray(W.T).astype(np.float16)
    w2t[:64, :64] = wt
    w2t[64:, 64:] = wt
    x16 = np.asarray(x, dtype=np.float16)
    in_maps = [
        {"x": np.ascontiguousarray(x16[c * B_PER_CORE:(c + 1) * B_PER_CORE]),
         "w2t": w2t}
        for c in range(N_CORES)
    ]
    res = run_bass_kernel_spmd(nc, in_maps, list(range(N_CORES)), trace=trace)
    out = np.concatenate([res.results[c]["out"] for c in range(N_CORES)], axis=0)
    return out.astype(np.float32), res


def kernel(x: np.ndarray, W: np.ndarray) -> np.ndarray:
    out, _ = _run(np.asarray(x, dtype=np.float32), np.asarray(W, dtype=np.float32))
    return out


# revision 41
# speedup vs baseline: 1.1687x; 1.1295x over previous
"""Trainium2 Bass kernel for nn_LNKillingRelu (fp16 pipeline).

Math (per batch b, channel g, point n; L=8 lie-algebra coords):
    d[b,g,:,n]  = sum_f W[g,f] * x[b,f,:,n]          (64x64 linear over channels)
    kf[b,g,n]   = x[b,g,:,n]^T K d[b,g,:,n]          (8x8 Killing bilinear form)
    out         = x + relu(kf) * d                    (broadcast kf over L)

K is symmetric & sparse: pairs (0<->6, 1<->7, 2<->3) plus [[2,1],[1,2]] on
coords (4,5).  We use kf = sum_l (Kx)[l] * d[l] with
(Kx) = [x6, x7, x3, x2, 2x4+x5, x4+2x5, x0, x1].

Everything on-device runs in fp16 (tolerance is 2e-2 scale-relative; fp16
keeps it ~1e-3): halves DMA bytes (the binding resource - all DMA transfers
serialize at ~360B/ns aggregate), doubles DVE throughput (2x packed mode),
and runs the PE at 1 cycle/row instead of 4.  Host converts fp32<->fp16.

Sharding: data-parallel over batch B=16 -> 2 batches per core (8 cores).
Per-core layout: partitions = (batch-pair, F=64) = 128; free = (L, n-chunk).
PE computes d with a block-diag(W^T, W^T) 128x128 stationary fp16 weight.
ACT drains PSUM->SBUF (fp32->fp16); DVE+GPSIMD share the Killing products,
tree-reduce, and the out = x + relu(kf)*d tail.

Software pipelining: relu(k-1) and the out-stage(k-1) are emitted inside
chunk k - relu goes on ACT *before* chunk k's PSUM drains (so ACT never
stalls on the DVE kf-chain), and the out-stage fills the DVE/Pool gap while
chunk k's D is still in the PE/ACT pipe.  Chunk sizes ramp 256 -> 512 -> 256
to shorten the pipeline head/tail; all tiles are allocated at max width and
column-sliced so the tile pools see a single tag.
"""

import numpy as np
from contextlib import ExitStack

import concourse.bass as bass
import concourse.bacc as bacc
import concourse.tile as tile
from concourse import mybir
from concourse.bass_utils import run_bass_kernel_spmd

B, F, L, N = 16, 64, 8, 4096
N_CORES = 8
B_PER_CORE = B // N_CORES  # 2

F16 = mybir.dt.float16
F32 = mybir.dt.float32
MULT = mybir.AluOpType.mult
ADD = mybir.AluOpType.add

NQ = 256  # psum quarter width (4 banks fp32 at L=8)


def _bcast_l(ap: bass.AP, l: int) -> bass.AP:
    """[128, NT] AP -> [128, l, NT] with a zero-stride middle dim."""
    return bass.AP(tensor=ap.tensor, offset=ap.offset,
                   ap=[ap.ap[0], [0, l], ap.ap[1]])


# default engine assignment: 'v' = DVE, 'p' = GPSIMD(Pool)
DEFAULT_ASSIGN = dict(pa='v', pb='v', pc='p', pd='v', pe='v',
                      xe0='p', xe1='p', xs=None, l1='v', l2='v', l3='v',
                      l1split=0, l1a='v', l1b='p',
                      osplit=3, pipe=1, ocolsplit=1, copygrain='pair',
                      pdsplit=0, relu='a', merge_pape=0, merge_pbpc=0)
DEFAULT_CHUNKS = (256, 256, 512, 512, 512, 512, 512, 512, 256, 256)


def _build(chunks=DEFAULT_CHUNKS, assign=None):
    asn = dict(DEFAULT_ASSIGN)
    if assign:
        asn.update(assign)
    assert sum(chunks) == N
    ntmax = max(chunks)
    nc = bacc.Bacc("TRN2", target_bir_lowering=False, debug=False,
                   num_devices=N_CORES)
    x = nc.dram_tensor("x", [B_PER_CORE, F, L, N], F16, kind="ExternalInput").ap()
    w2t = nc.dram_tensor("w2t", [128, 128], F16, kind="ExternalInput").ap()
    out = nc.dram_tensor("out", [B_PER_CORE, F, L, N], F16, kind="ExternalOutput").ap()

    xv = x.rearrange("b f l n -> (b f) l n")    # [128, 8, N]
    ov = out.rearrange("b f l n -> (b f) l n")

    def eng(key):
        return nc.vector if asn[key] == 'v' else nc.gpsimd

    with ExitStack() as ctx:
        tc = ctx.enter_context(tile.TileContext(nc))
        singles = ctx.enter_context(tc.tile_pool(name="singles", bufs=1))
        xpool = ctx.enter_context(tc.tile_pool(name="xp", bufs=asn.get('xbufs', 3)))
        dpool = ctx.enter_context(tc.tile_pool(name="dp", bufs=asn.get('dbufs', 3)))
        vpool = ctx.enter_context(tc.tile_pool(name="vp", bufs=asn.get('vbufs', 2)))
        tpool = ctx.enter_context(tc.tile_pool(name="tp", bufs=asn.get('tbufs', 2)))
        mpool = ctx.enter_context(tc.tile_pool(name="mp", bufs=asn.get('mbufs', 2)))
        opool = ctx.enter_context(tc.tile_pool(name="op", bufs=asn.get('obufs', 3)))
        rpool = ctx.enter_context(tc.tile_pool(name="rp", bufs=asn.get('rbufs', 3)))
        psum_bufs = {'pair': 4, 'half': 2}[asn['copygrain']]
        psum = ctx.enter_context(tc.tile_pool(name="ps", bufs=psum_bufs,
                                              space="PSUM"))

        w_sb = singles.tile([128, 128], F16)
        nc.sync.dma_start(out=w_sb[:], in_=w2t)
        zero16 = singles.tile([128, 1], F16, tag="zero16")
        nc.vector.memset(zero16[:], 0.0)

        pending = []  # chunks awaiting relu: (X, D, kf, n0, nt)
        ready = []    # chunks with r computed, awaiting their out-stage

        def emit_relu(item):
            X, D, kf, n0, nt = item
            r = rpool.tile([128, ntmax], F16, tag="r")
            if asn['relu'] == 'a':
                nc.scalar.activation(r[:, 0:nt], kf[:, 0:nt],
                                     mybir.ActivationFunctionType.Relu)
            elif asn['relu'] == 'v4':
                # fp16 [P,1] AP scalar keeps the 4x_2p DVE mode (a float
                # immediate would be fp32 and disqualify it)
                nc.vector.tensor_scalar_max(r[:, 0:nt], kf[:, 0:nt],
                                            zero16[:])
            else:
                e = nc.vector if asn['relu'] == 'v' else nc.gpsimd
                e.tensor_scalar_max(r[:, 0:nt], kf[:, 0:nt], 0.0)
            return (X, D, r, n0, nt)

        def emit_out_stage(item, mid_cb=None):
            X, D, r, n0, nt = item
            M = mpool.tile([128, L, ntmax], F16, tag="M")
            O = opool.tile([128, L, ntmax], F16, tag="O")
            s = asn['osplit']
            nh = nt // 2
            # Pool's share (and the out-DMA) go in column halves so the first
            # half can enter the serialized DMA queue while the second is
            # still being computed.  DVE's share is one op (it finishes early).
            colsplit = asn['ocolsplit'] and nh >= NQ
            if 0 < s < L:
                nc.vector.tensor_mul(M[:, 0:s, 0:nt], D[:, 0:s, 0:nt],
                                     _bcast_l(r[:, 0:nt], s))
                nc.vector.tensor_add(O[:, 0:s, 0:nt], X[:, 0:s, 0:nt],
                                     M[:, 0:s, 0:nt])
            # s==8: the whole out-stage runs on DVE inside the pieces loop
            peng = nc.vector if s == L else nc.gpsimd
            s0 = 0 if s == L else s
            pieces = [(0, nh), (nh, nt)] if colsplit else [(0, nt)]
            for i, (a, b) in enumerate(pieces):
                rb = _bcast_l(r[:, a:b], L - s0)
                peng.tensor_mul(M[:, s0:L, a:b], D[:, s0:L, a:b], rb)
                peng.tensor_add(O[:, s0:L, a:b], X[:, s0:L, a:b],
                                M[:, s0:L, a:b])
                nc.sync.dma_start(out=ov[:, :, n0 + a:n0 + b], in_=O[:, :, a:b])
                if i == 0 and mid_cb is not None:
                    # interleave the current chunk's Pool product between the
                    # two column halves so Pool's queue never head-of-line
                    # blocks the DVE tree behind the whole out-stage backlog
                    mid_cb()

        n0 = 0
        for nt in chunks:
            # in-DMA split by column halves: the first half's matmuls can
            # start while the second half is still transferring.
            X = xpool.tile([128, L, ntmax], F16, tag="X")
            npc = max(NQ, nt // asn.get('dmapieces', 2))
            for a in range(0, nt, npc):
                b = min(nt, a + npc)
                nc.sync.dma_start(out=X[:, :, a:b], in_=xv[:, :, n0 + a:n0 + b])

            # Xe = (2x4+x5, x4+2x5).  GPSIMD has no TensorScalarPtr, so the
            # Pool path uses plain adds via s = x4+x5; the DVE path can use
            # scalar_tensor_tensor directly (full rate but one op per plane).
            # Needs only X, fills the engines early.
            Xe = tpool.tile([128, 2, ntmax], F16, tag="Xe")
            if asn['xe0'] == 'v' and asn['xe1'] == 'v' and asn.get('xestt', 1):
                nc.vector.scalar_tensor_tensor(
                    Xe[:, 0, 0:nt], in0=X[:, 4, 0:nt], scalar=2.0,
                    in1=X[:, 5, 0:nt], op0=MULT, op1=ADD)
                nc.vector.scalar_tensor_tensor(
                    Xe[:, 1, 0:nt], in0=X[:, 5, 0:nt], scalar=2.0,
                    in1=X[:, 4, 0:nt], op0=MULT, op1=ADD)
            else:
                xs = tpool.tile([128, ntmax], F16, tag="xs")
                eng('xs' if asn['xs'] else 'xe0').tensor_add(
                    xs[:, 0:nt], X[:, 4, 0:nt], X[:, 5, 0:nt])
                eng('xe0').tensor_add(Xe[:, 0, 0:nt], xs[:, 0:nt], X[:, 4, 0:nt])
                eng('xe1').tensor_add(Xe[:, 1, 0:nt], xs[:, 0:nt], X[:, 5, 0:nt])

            # relu of the previous chunk: on ACT *before* this chunk's
            # copies, so it never blocks them (its input is long done).
            if pending:
                ready.append(emit_relu(pending.pop(0)))

            # D = W @ X.  PSUM tile height = ACT copy granularity g: finer
            # (g=2) unblocks downstream products earlier, coarser (g=4) cuts
            # ACT per-op overhead.  g*ntmax/256 banks per tile, 8 banks total.
            g = {'pair': 2, 'half': 4}[asn['copygrain']]
            D = dpool.tile([128, L, ntmax], F16, tag="D")
            for p in range(L // g):
                ps = psum.tile([128, g, ntmax], F32, tag="ps")
                for k in range(g):
                    nc.tensor.matmul(ps[:, k, 0:nt], lhsT=w_sb[:],
                                     rhs=X[:, g * p + k, 0:nt],
                                     start=True, stop=True)
                nc.scalar.copy(D[:, g * p:g * (p + 1), 0:nt], ps[:, :, 0:nt])

            # deferred out-stage of an earlier chunk (`opipe` chunks back):
            # ready to run, fills the DVE/Pool gap while this chunk's D is
            # in the PE/ACT pipe.
            V = vpool.tile([128, L, ntmax], F16, tag="V")
            pc_done = False

            def emit_pc(V=V, X=X, D=D, nt=nt):
                eng('pc').tensor_mul(V[:, 3, 0:nt], X[:, 2, 0:nt], D[:, 3, 0:nt])

            use_pc_mid = (asn.get('pc_mid', 0) and not asn['merge_pbpc']
                          and asn['pc'] == 'p')
            if ready and len(ready) + len(pending) >= asn['pipe']:
                emit_out_stage(ready.pop(0),
                               mid_cb=emit_pc if use_pc_mid else None)
                pc_done = use_pc_mid

            # V[l] = (Kx)[l] * D[l]

            def _blk4(t, l0, blkstride, lstride_sign=1):
                """4D AP over tile t: [part, blk=2 (stride blkstride),
                l=2 (stride +-ls), nt] starting at plane l0."""
                base = t[:]
                pdim = base.ap[0]
                ls = ntmax
                return bass.AP(tensor=base.tensor,
                               offset=base.offset + l0 * ls,
                               ap=[pdim, [blkstride * ls, 2],
                                   [lstride_sign * ls, 2], [1, nt]])

            if asn['merge_pape']:
                # V[{0,1}],V[{6,7}] = X[{6,7}],X[{0,1}] * D[{0,1}],D[{6,7}]
                eng('pa').tensor_tensor(
                    _blk4(V, 0, 6), _blk4(X, 6, -6), _blk4(D, 0, 6), MULT)
            else:
                eng('pa').tensor_mul(V[:, 0:2, 0:nt], X[:, 6:8, 0:nt],
                                     D[:, 0:2, 0:nt])
            if asn['merge_pbpc']:
                # V[2:4] = X[{3,2}] * D[2:4] via negative l-stride on X
                xrev = bass.AP(tensor=X[:].tensor,
                               offset=X[:].offset + 3 * ntmax,
                               ap=[X[:].ap[0], [-ntmax, 2], [1, nt]])
                eng('pb').tensor_tensor(V[:, 2:4, 0:nt], xrev,
                                        D[:, 2:4, 0:nt], MULT)
            else:
                eng('pb').tensor_mul(V[:, 2, 0:nt], X[:, 3, 0:nt], D[:, 2, 0:nt])
                if not pc_done:
                    emit_pc()
            if asn['pdsplit']:
                eng('pd').tensor_mul(V[:, 4, 0:nt], Xe[:, 0, 0:nt], D[:, 4, 0:nt])
                eng('pd').tensor_mul(V[:, 5, 0:nt], Xe[:, 1, 0:nt], D[:, 5, 0:nt])
            else:
                eng('pd').tensor_mul(V[:, 4:6, 0:nt], Xe[:, :, 0:nt],
                                     D[:, 4:6, 0:nt])
            if not asn['merge_pape']:
                eng('pe').tensor_mul(V[:, 6:8, 0:nt], X[:, 0:2, 0:nt],
                                     D[:, 6:8, 0:nt])

            # kf = sum_l V[l] via tree adds (tensor_tensor keeps the fp16 2x
            # DVE mode; tensor_reduce over strided l would run at full rate)
            T4 = tpool.tile([128, 4, ntmax], F16, tag="T4")
            T2 = tpool.tile([128, 2, ntmax], F16, tag="T2")
            kf = rpool.tile([128, ntmax], F16, tag="kf")
            if asn['l1split']:
                eng('l1a').tensor_add(T4[:, 0:2, 0:nt], V[:, 0:2, 0:nt],
                                      V[:, 4:6, 0:nt])
                eng('l1b').tensor_add(T4[:, 2:4, 0:nt], V[:, 2:4, 0:nt],
                                      V[:, 6:8, 0:nt])
            else:
                eng('l1').tensor_add(T4[:, :, 0:nt], V[:, 0:4, 0:nt],
                                     V[:, 4:8, 0:nt])
            eng('l2').tensor_add(T2[:, :, 0:nt], T4[:, 0:2, 0:nt], T4[:, 2:4, 0:nt])
            eng('l3').tensor_add(kf[:, 0:nt], T2[:, 0, 0:nt], T2[:, 1, 0:nt])

            if asn['pipe']:
                pending.append((X, D, kf, n0, nt))
            else:
                emit_out_stage(emit_relu((X, D, kf, n0, nt)))
            n0 += nt

        while pending:
            ready.append(emit_relu(pending.pop(0)))
        for item in ready:
            emit_out_stage(item)

    nc.finalize()
    return nc


_CACHED = {}
CFG = (DEFAULT_CHUNKS, ())


def _freeze(cfg):
    chunks, assign = cfg
    return (tuple(chunks), tuple(sorted(dict(assign).items())))


def _get_program(cfg=None):
    cfg = cfg or CFG
    key = _freeze(cfg)
    if key not in _CACHED:
        _CACHED[key] = _build(tuple(cfg[0]), dict(cfg[1]))
    return _CACHED[key]


def _run(x: np.ndarray, W: np.ndarray, trace: bool = False, cfg=None):
    nc = _get_program(cfg)
    w2t = np.zeros((128, 128), dtype=np.float16)
    wt = np.ascontiguousarray(W.T).astype(np.float16)
    w2t[:64, :64] = wt
    w2t[64:, 64:] = wt
    x16 = np.asarray(x, dtype=np.float16)
    in_maps = [
        {"x": np.ascontiguousarray(x16[c * B_PER_CORE:(c + 1) * B_PER_CORE]),
         "w2t": w2t}
        for c in range(N_CORES)
    ]
    res = run_bass_kernel_spmd(nc, in_maps, list(range(N_CORES)), trace=trace)
    out = np.concatenate([res.results[c]["out"] for c in range(N_CORES)], axis=0)
    return out.astype(np.float32), res


def kernel(x: np.ndarray, W: np.ndarray) -> np.ndarray:
    out, _ = _run(np.asarray(x, dtype=np.float32), np.asarray(W, dtype=np.float32))
    return out


# revision 47
# speedup vs baseline: 1.3416x; 1.1479x over previous
"""Trainium2 Bass kernel for nn_LNKillingRelu (fp16 pipeline).

Math (per batch b, channel g, point n; L=8 lie-algebra coords):
    d[b,g,:,n]  = sum_f W[g,f] * x[b,f,:,n]          (64x64 linear over channels)
    kf[b,g,n]   = x[b,g,:,n]^T K d[b,g,:,n]          (8x8 Killing bilinear form)
    out         = x + relu(kf) * d                    (broadcast kf over L)

K is symmetric & sparse: pairs (0<->6, 1<->7, 2<->3) plus [[2,1],[1,2]] on
coords (4,5).  We use kf = sum_l (Kx)[l] * d[l] with
(Kx) = [x6, x7, x3, x2, 2x4+x5, x4+2x5, x0, x1].

Everything on-device runs in fp16 (tolerance is 2e-2 scale-relative; fp16
keeps it ~1e-3): halves DMA bytes (the binding resource - all DMA transfers
serialize at ~360B/ns aggregate), doubles DVE throughput (2x packed mode),
and runs the PE at 1 cycle/row instead of 4.  Host converts fp32<->fp16.

Sharding: data-parallel over batch B=16 -> 2 batches per core (8 cores).
Per-core layout: partitions = (batch-pair, F=64) = 128; free = (L, n-chunk).
PE computes d with a block-diag(W^T, W^T) 128x128 stationary fp16 weight.
ACT drains PSUM->SBUF (fp32->fp16); DVE+GPSIMD share the Killing products,
tree-reduce, and the out = x + relu(kf)*d tail.

Software pipelining: relu(k-1) and the out-stage(k-1) are emitted inside
chunk k - relu goes on ACT *before* chunk k's PSUM drains (so ACT never
stalls on the DVE kf-chain), and the out-stage fills the DVE/Pool gap while
chunk k's D is still in the PE/ACT pipe.  Chunk sizes ramp 256 -> 512 -> 256
to shorten the pipeline head/tail; all tiles are allocated at max width and
column-sliced so the tile pools see a single tag.
"""

import numpy as np
from contextlib import ExitStack

import concourse.bass as bass
import concourse.bacc as bacc
import concourse.tile as tile
from concourse import mybir
from concourse.bass_utils import run_bass_kernel_spmd

B, F, L, N = 16, 64, 8, 4096
N_CORES = 8
B_PER_CORE = B // N_CORES  # 2

F16 = mybir.dt.float16
F32 = mybir.dt.float32
MULT = mybir.AluOpType.mult
ADD = mybir.AluOpType.add

NQ = 256  # psum quarter width (4 banks fp32 at L=8)


def _bcast_l(ap: bass.AP, l: int) -> bass.AP:
    """[128, NT] AP -> [128, l, NT] with a zero-stride middle dim."""
    return bass.AP(tensor=ap.tensor, offset=ap.offset,
                   ap=[ap.ap[0], [0, l], ap.ap[1]])


# default engine assignment: 'v' = DVE, 'p' = GPSIMD(Pool)
DEFAULT_ASSIGN = dict(pa='v', pb='v', pc='p', pd='v', pe='v',
                      xe0='p', xe1='p', xs=None, l1='v', l2='v', l3='v',
                      l1split=0, l1a='v', l1b='p',
                      osplit=3, pipe=1, ocolsplit=1, copygrain='pair',
                      pdsplit=0, relu='a', merge_pape=0, merge_pbpc=0,
                      treepe=0)
DEFAULT_CHUNKS = (256, 256, 512, 512, 512, 512, 512, 512, 256, 256)


def _build(chunks=DEFAULT_CHUNKS, assign=None):
    asn = dict(DEFAULT_ASSIGN)
    if assign:
        asn.update(assign)
    assert sum(chunks) == N
    ntmax = max(chunks)
    nc = bacc.Bacc("TRN2", target_bir_lowering=False, debug=False,
                   num_devices=N_CORES)
    x = nc.dram_tensor("x", [B_PER_CORE, F, L, N], F16, kind="ExternalInput").ap()
    w2t = nc.dram_tensor("w2t", [128, 128], F16, kind="ExternalInput").ap()
    if asn.get('treepe', 0):
        i128 = nc.dram_tensor("i128", [128, 128], F16, kind="ExternalInput").ap()
    out = nc.dram_tensor("out", [B_PER_CORE, F, L, N], F16, kind="ExternalOutput").ap()

    xv = x.rearrange("b f l n -> (b f) l n")    # [128, 8, N]
    ov = out.rearrange("b f l n -> (b f) l n")

    def eng(key):
        return nc.vector if asn[key] == 'v' else nc.gpsimd

    with ExitStack() as ctx:
        tc = ctx.enter_context(tile.TileContext(nc))
        singles = ctx.enter_context(tc.tile_pool(name="singles", bufs=1))
        xpool = ctx.enter_context(tc.tile_pool(name="xp", bufs=asn.get('xbufs', 3)))
        dpool = ctx.enter_context(tc.tile_pool(name="dp", bufs=asn.get('dbufs', 3)))
        vpool = ctx.enter_context(tc.tile_pool(name="vp", bufs=asn.get('vbufs', 2)))
        tpool = ctx.enter_context(tc.tile_pool(name="tp", bufs=asn.get('tbufs', 2)))
        mpool = ctx.enter_context(tc.tile_pool(name="mp", bufs=asn.get('mbufs', 2)))
        opool = ctx.enter_context(tc.tile_pool(name="op", bufs=asn.get('obufs', 3)))
        rpool = ctx.enter_context(tc.tile_pool(name="rp", bufs=asn.get('rbufs', 3)))
        if asn['treepe']:
            # D-pair tiles 3x2 banks + kf accumulators 2x1 bank = 8 banks
            psum_bufs = 3
            kpsum = ctx.enter_context(tc.tile_pool(name="kps", bufs=2,
                                                   space="PSUM"))
        else:
            psum_bufs = {'pair': 4, 'half': 2}[asn['copygrain']]
        psum = ctx.enter_context(tc.tile_pool(name="ps", bufs=psum_bufs,
                                              space="PSUM"))

        w_sb = singles.tile([128, 128], F16)
        nc.sync.dma_start(out=w_sb[:], in_=w2t)
        zero16 = singles.tile([128, 1], F16, tag="zero16")
        nc.vector.memset(zero16[:], 0.0)
        if asn['treepe']:
            i_sb = singles.tile([128, 128], F16, tag="i128")
            nc.sync.dma_start(out=i_sb[:], in_=i128)

        pending = []  # chunks awaiting relu: (X, D, kf, n0, nt)
        ready = []    # chunks with r computed, awaiting their out-stage

        def emit_relu(item):
            X, D, kf, n0, nt = item
            r = rpool.tile([128, ntmax], F16, tag="r")
            if asn['relu'] == 'a':
                nc.scalar.activation(r[:, 0:nt], kf[:, 0:nt],
                                     mybir.ActivationFunctionType.Relu)
            elif asn['relu'] == 'v4':
                # fp16 [P,1] AP scalar keeps the 4x_2p DVE mode (a float
                # immediate would be fp32 and disqualify it)
                nc.vector.tensor_scalar_max(r[:, 0:nt], kf[:, 0:nt],
                                            zero16[:])
            else:
                e = nc.vector if asn['relu'] == 'v' else nc.gpsimd
                e.tensor_scalar_max(r[:, 0:nt], kf[:, 0:nt], 0.0)
            return (X, D, r, n0, nt)

        def emit_out_stage(item, mid_cb=None):
            X, D, r, n0, nt = item
            M = mpool.tile([128, L, ntmax], F16, tag="M")
            O = opool.tile([128, L, ntmax], F16, tag="O")
            s = asn['osplit']
            nh = nt // 2
            # Pool's share (and the out-DMA) go in column halves so the first
            # half can enter the serialized DMA queue while the second is
            # still being computed.  DVE's share is one op (it finishes early).
            colsplit = asn['ocolsplit'] and nh >= NQ
            if 0 < s < L:
                nc.vector.tensor_mul(M[:, 0:s, 0:nt], D[:, 0:s, 0:nt],
                                     _bcast_l(r[:, 0:nt], s))
                nc.vector.tensor_add(O[:, 0:s, 0:nt], X[:, 0:s, 0:nt],
                                     M[:, 0:s, 0:nt])
            # s==8: the whole out-stage runs on DVE inside the pieces loop
            peng = nc.vector if s == L else nc.gpsimd
            s0 = 0 if s == L else s
            pieces = [(0, nh), (nh, nt)] if colsplit else [(0, nt)]
            for i, (a, b) in enumerate(pieces):
                rb = _bcast_l(r[:, a:b], L - s0)
                peng.tensor_mul(M[:, s0:L, a:b], D[:, s0:L, a:b], rb)
                peng.tensor_add(O[:, s0:L, a:b], X[:, s0:L, a:b],
                                M[:, s0:L, a:b])
                nc.sync.dma_start(out=ov[:, :, n0 + a:n0 + b], in_=O[:, :, a:b])
                if i == 0 and mid_cb is not None:
                    # interleave the current chunk's Pool product between the
                    # two column halves so Pool's queue never head-of-line
                    # blocks the DVE tree behind the whole out-stage backlog
                    mid_cb()

        n0 = 0
        for nt in chunks:
            # in-DMA split by column halves: the first half's matmuls can
            # start while the second half is still transferring.
            X = xpool.tile([128, L, ntmax], F16, tag="X")
            npc = max(NQ, nt // asn.get('dmapieces', 2))
            for a in range(0, nt, npc):
                b = min(nt, a + npc)
                nc.sync.dma_start(out=X[:, :, a:b], in_=xv[:, :, n0 + a:n0 + b])

            # Xe = (2x4+x5, x4+2x5).  GPSIMD has no TensorScalarPtr, so the
            # Pool path uses plain adds via s = x4+x5; the DVE path can use
            # scalar_tensor_tensor directly (full rate but one op per plane).
            # Needs only X, fills the engines early.
            Xe = tpool.tile([128, 2, ntmax], F16, tag="Xe")
            if asn['xe0'] == 'v' and asn['xe1'] == 'v' and asn.get('xestt', 1):
                nc.vector.scalar_tensor_tensor(
                    Xe[:, 0, 0:nt], in0=X[:, 4, 0:nt], scalar=2.0,
                    in1=X[:, 5, 0:nt], op0=MULT, op1=ADD)
                nc.vector.scalar_tensor_tensor(
                    Xe[:, 1, 0:nt], in0=X[:, 5, 0:nt], scalar=2.0,
                    in1=X[:, 4, 0:nt], op0=MULT, op1=ADD)
            else:
                xs = tpool.tile([128, ntmax], F16, tag="xs")
                eng('xs' if asn['xs'] else 'xe0').tensor_add(
                    xs[:, 0:nt], X[:, 4, 0:nt], X[:, 5, 0:nt])
                eng('xe0').tensor_add(Xe[:, 0, 0:nt], xs[:, 0:nt], X[:, 4, 0:nt])
                eng('xe1').tensor_add(Xe[:, 1, 0:nt], xs[:, 0:nt], X[:, 5, 0:nt])

            # relu of the previous chunk: on ACT *before* this chunk's
            # copies, so it never blocks them (its input is long done).
            if pending:
                ready.append(emit_relu(pending.pop(0)))

            # D = W @ X.  PSUM tile height = ACT copy granularity g: finer
            # (g=2) unblocks downstream products earlier, coarser (g=4) cuts
            # ACT per-op overhead.  g*ntmax/256 banks per tile, 8 banks total.
            g = {'pair': 2, 'half': 4}[asn['copygrain']]
            D = dpool.tile([128, L, ntmax], F16, tag="D")
            for p in range(L // g):
                ps = psum.tile([128, g, ntmax], F32, tag="ps")
                for k in range(g):
                    nc.tensor.matmul(ps[:, k, 0:nt], lhsT=w_sb[:],
                                     rhs=X[:, g * p + k, 0:nt],
                                     start=True, stop=True)
                nc.scalar.copy(D[:, g * p:g * (p + 1), 0:nt], ps[:, :, 0:nt])

            # deferred out-stage of an earlier chunk (`opipe` chunks back):
            # ready to run, fills the DVE/Pool gap while this chunk's D is
            # in the PE/ACT pipe.
            V = vpool.tile([128, L, ntmax], F16, tag="V")
            pc_done = False

            def emit_pc(V=V, X=X, D=D, nt=nt):
                eng('pc').tensor_mul(V[:, 3, 0:nt], X[:, 2, 0:nt], D[:, 3, 0:nt])

            use_pc_mid = (asn.get('pc_mid', 0) and not asn['merge_pbpc']
                          and asn['pc'] == 'p')
            if ready and len(ready) + len(pending) >= asn['pipe']:
                emit_out_stage(ready.pop(0),
                               mid_cb=emit_pc if use_pc_mid else None)
                pc_done = use_pc_mid

            # V[l] = (Kx)[l] * D[l]

            def _blk4(t, l0, blkstride, lstride_sign=1):
                """4D AP over tile t: [part, blk=2 (stride blkstride),
                l=2 (stride +-ls), nt] starting at plane l0."""
                base = t[:]
                pdim = base.ap[0]
                ls = ntmax
                return bass.AP(tensor=base.tensor,
                               offset=base.offset + l0 * ls,
                               ap=[pdim, [blkstride * ls, 2],
                                   [lstride_sign * ls, 2], [1, nt]])

            if asn['merge_pape']:
                # V[{0,1}],V[{6,7}] = X[{6,7}],X[{0,1}] * D[{0,1}],D[{6,7}]
                eng('pa').tensor_tensor(
                    _blk4(V, 0, 6), _blk4(X, 6, -6), _blk4(D, 0, 6), MULT)
            else:
                eng('pa').tensor_mul(V[:, 0:2, 0:nt], X[:, 6:8, 0:nt],
                                     D[:, 0:2, 0:nt])
            if asn['merge_pbpc']:
                # V[2:4] = X[{3,2}] * D[2:4] via negative l-stride on X
                xrev = bass.AP(tensor=X[:].tensor,
                               offset=X[:].offset + 3 * ntmax,
                               ap=[X[:].ap[0], [-ntmax, 2], [1, nt]])
                eng('pb').tensor_tensor(V[:, 2:4, 0:nt], xrev,
                                        D[:, 2:4, 0:nt], MULT)
            else:
                eng('pb').tensor_mul(V[:, 2, 0:nt], X[:, 3, 0:nt], D[:, 2, 0:nt])
                if not pc_done:
                    emit_pc()
            if asn['pdsplit']:
                eng('pd').tensor_mul(V[:, 4, 0:nt], Xe[:, 0, 0:nt], D[:, 4, 0:nt])
                eng('pd').tensor_mul(V[:, 5, 0:nt], Xe[:, 1, 0:nt], D[:, 5, 0:nt])
            else:
                eng('pd').tensor_mul(V[:, 4:6, 0:nt], Xe[:, :, 0:nt],
                                     D[:, 4:6, 0:nt])
            if not asn['merge_pape']:
                eng('pe').tensor_mul(V[:, 6:8, 0:nt], X[:, 0:2, 0:nt],
                                     D[:, 6:8, 0:nt])

            # kf = sum_l V[l] via tree adds (tensor_tensor keeps the fp16 2x
            # DVE mode; tensor_reduce over strided l would run at full rate)
            if asn['treepe']:
                # kf = sum_l V[l] on the PE: 8 accumulating identity matmuls
                # into one PSUM bank.  Frees the DVE/Pool tree entirely; the
                # deferred relu reads the PSUM accumulator directly.
                kf = kpsum.tile([128, ntmax], F32, tag="kf_ps")
                for l in range(L):
                    nc.tensor.matmul(kf[:, 0:nt], lhsT=i_sb[:],
                                     rhs=V[:, l, 0:nt],
                                     start=(l == 0), stop=(l == L - 1))
                if asn['pipe']:
                    pending.append((X, D, kf, n0, nt))
                else:
                    emit_out_stage(emit_relu((X, D, kf, n0, nt)))
                n0 += nt
                continue

            T4 = tpool.tile([128, 4, ntmax], F16, tag="T4")
            T2 = tpool.tile([128, 2, ntmax], F16, tag="T2")
            kf = rpool.tile([128, ntmax], F16, tag="kf")
            if asn['l1split']:
                eng('l1a').tensor_add(T4[:, 0:2, 0:nt], V[:, 0:2, 0:nt],
                                      V[:, 4:6, 0:nt])
                eng('l1b').tensor_add(T4[:, 2:4, 0:nt], V[:, 2:4, 0:nt],
                                      V[:, 6:8, 0:nt])
            else:
                eng('l1').tensor_add(T4[:, :, 0:nt], V[:, 0:4, 0:nt],
                                     V[:, 4:8, 0:nt])
            eng('l2').tensor_add(T2[:, :, 0:nt], T4[:, 0:2, 0:nt], T4[:, 2:4, 0:nt])
            eng('l3').tensor_add(kf[:, 0:nt], T2[:, 0, 0:nt], T2[:, 1, 0:nt])

            if asn['pipe']:
                pending.append((X, D, kf, n0, nt))
            else:
                emit_out_stage(emit_relu((X, D, kf, n0, nt)))
            n0 += nt

        while pending:
            ready.append(emit_relu(pending.pop(0)))
        for item in ready:
            emit_out_stage(item)

    nc.finalize()
    return nc


_CACHED = {}
CFG = (DEFAULT_CHUNKS, ())


def _freeze(cfg):
    chunks, assign = cfg
    return (tuple(chunks), tuple(sorted(dict(assign).items())))


def _get_program(cfg=None):
    cfg = cfg or CFG
    key = _freeze(cfg)
    if key not in _CACHED:
        _CACHED[key] = _build(tuple(cfg[0]), dict(cfg[1]))
    return _CACHED[key]


def _run(x: np.ndarray, W: np.ndarray, trace: bool = False, cfg=None):
    nc = _get_program(cfg)
    w2t = np.zeros((128, 128), dtype=np.float16)
    wt = np.ascontiguousarray(W.T).astype(np.float16)
    w2t[:64, :64] = wt
    w2t[64:, 64:] = wt
    x16 = np.asarray(x, dtype=np.float16)
    extra = {}
    asn = dict(dict(DEFAULT_ASSIGN), **dict((cfg or CFG)[1]))
    if asn.get('treepe', 0):
        extra["i128"] = np.eye(128, dtype=np.float16)
    in_maps = [
        {"x": np.ascontiguousarray(x16[c * B_PER_CORE:(c + 1) * B_PER_CORE]),
         "w2t": w2t, **extra}
        for c in range(N_CORES)
    ]
    res = run_bass_kernel_spmd(nc, in_maps, list(range(N_CORES)), trace=trace)
    out = np.concatenate([res.results[c]["out"] for c in range(N_CORES)], axis=0)
    return out.astype(np.float32), res


def kernel(x: np.ndarray, W: np.ndarray) -> np.ndarray:
    out, _ = _run(np.asarray(x, dtype=np.float32), np.asarray(W, dtype=np.float32))
    return out
